# revision 2
# baseline (speedup 1.0000x reference)
"""GraphSpmv + L1 loss on 8 trn2 cores.

Design: node-sharded dense rounds (degree-sorted node permutation makes every
round a prefix => no scatter anywhere). Per core the only irregular op is the
d[src] gather, done as dma_gather of 64-f32 windows (256B descriptors) followed
by an on-chip select: mask = (iota64 == rel), g = reduce_add(mask * window).
Then Ad accumulates densely per round, masked L1 is reduced on-chip and only a
single scalar per core leaves the device. Host sums 8 partials / N.
"""
import sys
sys.path.insert(0, "/opt/trn_rl_repo")
import numpy as np

N_NODES = 500_000
N_EDGES = 16_000_000
N_CORES = 8
P = 128
ES = 64                      # window elems (f32) per descriptor = 256B
CHUNKC = 64                  # slot columns per dma_gather call (64*128=8192 idxs)
NI_C = P * CHUNKC            # 8192
S16 = NI_C // 16             # 512 idx cols per chunk (wrapped int16)
NBLK = (N_NODES + ES - 1) // ES   # 7813 window rows
NQ = N_NODES // N_CORES      # 62500 nodes per core
NODECOL = -(-NQ // P)        # 489

_RUNNER = None
_SCALE = None                # host-side final divide


def _build(totcol, nchunk, rounds, adw):
    """rounds: list of (chunk_start, chunk_end, ad_col_offset_of_chunk0)."""
    import concourse.bass as bass
    import concourse.bacc as bacc
    import concourse.mybir as mybir
    from concourse import library_config as lc

    nc = bacc.Bacc(None, target_bir_lowering=False)
    dtab = nc.dram_tensor("dtab", [NBLK, ES], mybir.dt.float32, kind="ExternalInput")
    wid = nc.dram_tensor("wid", [P, nchunk * S16], mybir.dt.int16, kind="ExternalInput")
    rel = nc.dram_tensor("rel", [P, totcol], mybir.dt.float32, kind="ExternalInput")
    vals = nc.dram_tensor("vals", [P, totcol], mybir.dt.float32, kind="ExternalInput")
    resid = nc.dram_tensor("resid", [P, adw], mybir.dt.float32, kind="ExternalInput")
    maskf = nc.dram_tensor("maskf", [P, adw], mybir.dt.float32, kind="ExternalInput")
    iota = nc.dram_tensor("iota", [P, CHUNKC * ES], mybir.dt.float32, kind="ExternalInput")
    lsum = nc.dram_tensor("lsum", [1, 1], mybir.dt.float32, kind="ExternalOutput")

    # chunk -> ad column offset
    ad_off = []
    for (c0, c1, off0) in rounds:
        for t in range(c0, c1):
            ad_off.append(off0 + (t - c0) * CHUNKC)
    assert len(ad_off) == nchunk

    from contextlib import ExitStack
    with ExitStack() as _st:
        block = _st.enter_context(nc.Block())
        s_c = _st.enter_context(nc.semaphore("s_c"))
        s_idx = _st.enter_context(nc.semaphore("s_idx"))
        s_rv = _st.enter_context(nc.semaphore("s_rv"))
        s_g = _st.enter_context(nc.semaphore("s_g"))
        s_dve = _st.enter_context(nc.semaphore("s_dve"))
        s_row = _st.enter_context(nc.semaphore("s_row"))
        s_mm = _st.enter_context(nc.semaphore("s_mm"))
        s_fin = _st.enter_context(nc.semaphore("s_fin"))
        s_out = _st.enter_context(nc.semaphore("s_out"))
        idx_sb = _st.enter_context(nc.sbuf_tensor("idx_sb", [P, 2, S16], mybir.dt.int16))
        win_sb = _st.enter_context(nc.sbuf_tensor("win_sb", [P, 2, CHUNKC, ES], mybir.dt.float32))
        rel_sb = _st.enter_context(nc.sbuf_tensor("rel_sb", [P, 2, CHUNKC], mybir.dt.float32))
        val_sb = _st.enter_context(nc.sbuf_tensor("val_sb", [P, 2, CHUNKC], mybir.dt.float32))
        mask_sb = _st.enter_context(nc.sbuf_tensor("mask_sb", [P, CHUNKC, ES], mybir.dt.float32))
        gsum_sb = _st.enter_context(nc.sbuf_tensor("gsum_sb", [P, CHUNKC], mybir.dt.float32))
        ad_sb = _st.enter_context(nc.sbuf_tensor("ad_sb", [P, adw], mybir.dt.float32))
        res_sb = _st.enter_context(nc.sbuf_tensor("res_sb", [P, adw], mybir.dt.float32))
        mk_sb = _st.enter_context(nc.sbuf_tensor("mk_sb", [P, adw], mybir.dt.float32))
        io_sb = _st.enter_context(nc.sbuf_tensor("io_sb", [P, CHUNKC, ES], mybir.dt.float32))
        ones_sb = _st.enter_context(nc.sbuf_tensor("ones_sb", [P, 1], mybir.dt.float32))
        row_sb = _st.enter_context(nc.sbuf_tensor("row_sb", [P, 1], mybir.dt.float32))
        out_sb = _st.enter_context(nc.sbuf_tensor("out_sb", [1, 1], mybir.dt.float32))
        ps = _st.enter_context(nc.psum_tensor("ps", [1, 1], mybir.dt.float32))
        @block.sync
        def _(sync):
            sync.dma_start(res_sb[:, :], resid.ap()).then_inc(s_c, 16)
            sync.dma_start(mk_sb[:, :], maskf.ap()).then_inc(s_c, 16)
            sync.dma_start(io_sb[:, :, :].rearrange("p a b -> p (a b)"), iota.ap()).then_inc(s_c, 16)
            for t in range(nchunk):
                if t >= 2:
                    sync.wait_ge(s_g, 16 * (t - 1))   # idx buf reused after gather t-2
                    sync.wait_ge(s_dve, t - 1)        # rel/val bufs consumed by DVE t-2
                sync.dma_start(idx_sb[:, t % 2, :],
                               wid.ap()[:, t * S16:(t + 1) * S16]).then_inc(s_idx, 16)
                sync.dma_start(rel_sb[:, t % 2, :],
                               rel.ap()[:, t * CHUNKC:(t + 1) * CHUNKC]).then_inc(s_rv, 16)
                sync.dma_start(val_sb[:, t % 2, :],
                               vals.ap()[:, t * CHUNKC:(t + 1) * CHUNKC]).then_inc(s_rv, 16)
            sync.wait_ge(s_fin, 1)
            sync.dma_start(lsum.ap(), out_sb[:, :]).then_inc(s_out, 16)
            sync.wait_ge(s_out, 16)

        @block.gpsimd
        def _(g):
            g.load_library(lc.mlp)
            for t in range(nchunk):
                g.wait_ge(s_idx, 16 * (t + 1))
                if t >= 2:
                    g.wait_ge(s_dve, t - 1)           # win buf consumed by DVE t-2
                g.dma_gather(
                    out_ap=win_sb[:, t % 2, :, :],
                    in_ap=dtab.ap(),
                    idxs_ap=idx_sb[:, t % 2, :],
                    num_idxs=NI_C,
                    num_idxs_reg=NI_C,
                    elem_size=ES,
                    single_packet=False,
                ).then_inc(s_g, 16)

        @block.vector
        def _(v):
            v.memset(ad_sb[:, :], 0)
            v.memset(ones_sb[:, :], 1.0)
            v.wait_ge(s_c, 48)
            for t in range(nchunk):
                v.wait_ge(s_g, 16 * (t + 1))
                v.wait_ge(s_rv, 32 * (t + 1))
                b = t % 2
                # mask = (iota == rel)
                v.tensor_tensor(
                    out=mask_sb[:, :, :],
                    in0=io_sb[:, :, :],
                    in1=rel_sb[:, b, :].rearrange("p (c o) -> p c o", o=1).to_broadcast([P, CHUNKC, ES]),
                    op=mybir.AluOpType.is_equal,
                )
                # mask *= window
                v.tensor_tensor(
                    out=mask_sb[:, :, :], in0=mask_sb[:, :, :],
                    in1=win_sb[:, b, :, :], op=mybir.AluOpType.mult,
                )
                # g = sum over window
                v.tensor_reduce(
                    out=gsum_sb[:, :], in_=mask_sb[:, :, :],
                    axis=mybir.AxisListType.X, op=mybir.AluOpType.add,
                )
                # g *= vals
                v.tensor_tensor(
                    out=gsum_sb[:, :], in0=gsum_sb[:, :],
                    in1=val_sb[:, b, :], op=mybir.AluOpType.mult,
                )
                # Ad += g
                a = ad_off[t]
                v.tensor_tensor(
                    out=ad_sb[:, a:a + CHUNKC], in0=ad_sb[:, a:a + CHUNKC],
                    in1=gsum_sb[:, :], op=mybir.AluOpType.add,
                ).then_inc(s_dve, 1)
            # masked L1
            v.tensor_tensor(out=ad_sb[:, :], in0=ad_sb[:, :], in1=mk_sb[:, :],
                            op=mybir.AluOpType.mult)
            v.tensor_tensor(out=ad_sb[:, :], in0=ad_sb[:, :], in1=res_sb[:, :],
                            op=mybir.AluOpType.subtract)
            v.tensor_reduce(out=row_sb[:, :], in_=ad_sb[:, :],
                            axis=mybir.AxisListType.X,
                            op=mybir.AluOpType.add,
                            apply_absolute_value=True).then_inc(s_row, 1)
            v.wait_ge(s_mm, 1)
            v.tensor_scalar_add(out_sb[:, :], ps[:, :], 0.0).then_inc(s_fin, 1)

        @block.tensor
        def _(te):
            te.wait_ge(s_row, 1)
            te.matmul(out=ps[:, :], lhsT=ones_sb[:, :], rhs=row_sb[:, :],
                      start=True, stop=True).then_inc(s_mm, 1)

    nc.finalize()
    return nc


# ---- embedded SPMD runner ----
import time
import jax
from jax.sharding import Mesh, PartitionSpec
from jax.experimental.shard_map import shard_map

import concourse.bass as bass
import concourse.mybir as mybir
from concourse import bass2jax
from concourse.bass2jax import _bass_exec_p, install_neuronx_cc_hook, partition_id_tensor


class SpmdRunner:
    def __init__(self, nc, n_cores=8):
        install_neuronx_cc_hook()
        self.nc = nc
        self.n_cores = n_cores
        assert nc.dbg_addr is None or not nc.dbg_callbacks
        partition_name = nc.partition_id_tensor.name if nc.partition_id_tensor else None
        in_names, out_names, out_avals, zero_outs = [], [], [], []
        for alloc in nc.m.functions[0].allocations:
            if not isinstance(alloc, mybir.MemoryLocationSet):
                continue
            name = alloc.memorylocations[0].name
            if alloc.kind == "ExternalInput":
                if name != partition_name and name != (nc.dbg_addr.name if nc.dbg_addr else None):
                    in_names.append(name)
            elif alloc.kind == "ExternalOutput":
                out_names.append(name)
                shape = tuple(alloc.tensor_shape)
                dtype = mybir.dt.np(alloc.dtype)
                out_avals.append(jax.core.ShapedArray(shape, dtype))
                zero_outs.append(np.zeros(shape, dtype))
        self.in_names, self.out_names = in_names, out_names
        self.out_avals, self.zero_outs = out_avals, zero_outs
        n_params, n_outs = len(in_names), len(out_avals)
        self.n_params = n_params

        all_in_names = list(in_names) + list(out_names)
        if nc.dbg_addr is not None:
            self.dbg_name = nc.dbg_addr.name
        else:
            self.dbg_name = None
        if partition_name is not None:
            all_in_names.append(partition_name)

        def _body(*args):
            operands = list(args)
            if partition_name is not None:
                operands.append(partition_id_tensor())
            outs = _bass_exec_p.bind(
                *operands,
                out_avals=tuple(out_avals),
                in_names=tuple(all_in_names),
                out_names=tuple(out_names),
                lowering_input_output_aliases=(),
                sim_require_finite=True,
                sim_require_nnan=True,
                nc=nc,
            )
            return tuple(outs)

        devices = jax.devices()[:n_cores]
        self.mesh = Mesh(np.asarray(devices), ("core",))
        in_specs = (PartitionSpec("core"),) * (n_params + n_outs)
        out_specs = (PartitionSpec("core"),) * n_outs
        self.fn = jax.jit(
            shard_map(_body, mesh=self.mesh, in_specs=in_specs,
                      out_specs=out_specs, check_rep=False),
            keep_unused=True,
        )
        self._cached_dev_in = None

    def put_inputs(self, in_maps):
        concat = [
            np.concatenate([np.asarray(in_maps[c][n]) for c in range(self.n_cores)], axis=0)
            for n in self.in_names
        ]
        concat += [
            np.zeros((self.n_cores * z.shape[0], *z.shape[1:]), z.dtype)
            for z in self.zero_outs
        ]
        self._cached_dev_in = jax.device_put(concat)
        return self._cached_dev_in

    def run(self, dev_in=None):
        dev_in = dev_in if dev_in is not None else self._cached_dev_in
        outs = self.fn(*dev_in)
        jax.block_until_ready(outs)
        return outs

    def results(self, outs):
        res = []
        for c in range(self.n_cores):
            m = {}
            for i, name in enumerate(self.out_names):
                a = np.asarray(outs[i]).reshape(self.n_cores, *self.out_avals[i].shape)
                m[name] = a[c]
            res.append(m)
        return res

    def time_runs(self, reps=5):
        ts = []
        for _ in range(reps):
            t0 = time.perf_counter()
            self.run()
            ts.append(time.perf_counter() - t0)
        return min(ts), ts


def _prep(d, edge_index, matrix_values, mask, residual):
    dst = np.asarray(edge_index[0], dtype=np.int64)
    src = np.asarray(edge_index[1], dtype=np.int32)
    mv = np.asarray(matrix_values, dtype=np.float32)
    d = np.asarray(d, dtype=np.float32)
    N = N_NODES

    deg = np.bincount(dst, minlength=N)
    order = np.argsort(-deg, kind="stable")
    rank = np.empty(N, np.int64)
    rank[order] = np.arange(N)

    r8 = rank[dst]
    core_e = r8 % N_CORES
    q_e = r8 // N_CORES
    ekey = core_e * (2 ** 32) + q_e
    eorder = np.argsort(ekey, kind="stable")
    core_s = core_e[eorder]
    q_s = q_e[eorder]
    src_s = src[eorder]
    mv_s = mv[eorder]

    gid = ekey[eorder]
    changes = np.r_[True, gid[1:] != gid[:-1]]
    starts = np.flatnonzero(changes)
    group_of = np.cumsum(changes) - 1
    j_s = np.arange(len(gid)) - starts[group_of]

    # per-core degree profile -> global round widths (cols, padded to CHUNKC)
    degq = np.zeros((N_CORES, NQ), np.int64)
    for c in range(N_CORES):
        nodes_c = order[c::N_CORES]
        degq[c, :len(nodes_c)] = deg[nodes_c]
    R = int(degq.max())
    Lmax = np.zeros(R, np.int64)
    for c in range(N_CORES):
        dq = degq[c]
        for r in range(R):
            Lmax[r] = max(Lmax[r], np.searchsorted(-dq, -r, side="right"))
    W = -(-Lmax // P)                      # cols per round
    W = (-(-W // CHUNKC) * CHUNKC).astype(np.int64)
    B = np.r_[0, np.cumsum(W)]
    totcol = int(B[-1])
    nchunk = totcol // CHUNKC
    adw = max(NODECOL, int(W.max()))

    rounds = []
    for r in range(R):
        c0 = int(B[r]) // CHUNKC
        c1 = int(B[r + 1]) // CHUNKC
        rounds.append((c0, c1, 0))

    offsA = np.zeros((N_CORES, P, totcol), np.int32)
    valsA = np.zeros((N_CORES, P, totcol), np.float32)
    p_s = (q_s % P).astype(np.int64)
    col_s = B[j_s] + q_s // P
    flat = (core_s * P + p_s) * totcol + col_s
    valsA.reshape(-1)[flat] = mv_s
    offsA.reshape(-1)[flat] = src_s

    residA = np.zeros((N_CORES, P, adw), np.float32)
    maskA = np.zeros((N_CORES, P, adw), np.float32)
    resid = np.asarray(residual, dtype=np.float32)
    mk = np.asarray(mask).astype(np.float32)
    for c in range(N_CORES):
        nodes_c = order[c::N_CORES]
        qv = np.arange(len(nodes_c))
        residA[c, qv % P, qv // P] = resid[nodes_c]
        maskA[c, qv % P, qv // P] = mk[nodes_c]

    dpad = np.zeros(NBLK * ES, np.float32)
    dpad[:N] = d
    dtab = dpad.reshape(NBLK, ES)
    iota = np.tile(np.arange(ES, dtype=np.float32), (P, CHUNKC))

    in_maps = []
    for c in range(N_CORES):
        wids = (offsA[c] >> 6).astype(np.int16)     # [P, totcol]
        relC = (offsA[c] & 63).astype(np.float32)
        # idx stream per chunk: i = cloc*128 + p ; wrapped [16, S16] replicated
        widcols = []
        for t in range(nchunk):
            blockw = wids[:, t * CHUNKC:(t + 1) * CHUNKC]    # [P, 64]
            arr = blockw.T.ravel()                           # i = c*128+p order
            wrap = arr.reshape(S16, 16).T                    # [16, S16]
            widcols.append(np.tile(wrap, (8, 1)))            # [128, S16]
        in_maps.append({
            "dtab": dtab,
            "wid": np.concatenate(widcols, axis=1),
            "rel": relC,
            "vals": valsA[c],
            "resid": residA[c],
            "maskf": maskA[c],
            "iota": iota,
        })
    return in_maps, totcol, nchunk, rounds, adw


def kernel(d, edge_index, matrix_values, mask, residual):
    global _RUNNER, _SCALE
    in_maps, totcol, nchunk, rounds, adw = _prep(
        d, edge_index, matrix_values, mask, residual)
    if _RUNNER is None:
        nc = _build(totcol, nchunk, rounds, adw)
        _RUNNER = SpmdRunner(nc, N_CORES)
    _RUNNER.put_inputs(in_maps)
    outs = _RUNNER.run()
    res = _RUNNER.results(outs)
    total = sum(float(res[c]["lsum"][0, 0]) for c in range(N_CORES))
    return np.float32(total / N_NODES)


def _get_runner():
    return _RUNNER
